# revision 4
# baseline (speedup 1.0000x reference)
"""Trainium2 Bass kernel for nn_PoDiNN_85727547228676 (dense_mlp).

Computation (see reference): two element_grad (grad of per-group 1->H->H->1
tanh MLP) on q_k / v_m, coupling matmuls, two forward group-MLPs, final
coupling to outputs f_k/f_m.

Strategy: pure data parallel over batch (4096 -> 8 x 512). All activations
live as [hidden=128 partitions, batch=512 free] tiles; per-group H x H
matmuls run on the TensorEngine with pre-transposed stationaries. Per-group
partition reductions use a sliding ones-window stationary accumulating into
one PSUM bank (row g per group). Host does layout prep (transposes, masked
W1 expansion, W2*W3 folding) in numpy.
"""
import numpy as np
from contextlib import ExitStack

import concourse.bacc as bacc
import concourse.bass as bass
import concourse.tile as tile
from concourse import mybir
from concourse.bass_utils import run_bass_kernel_spmd

N_CORES = 8
B = 4096
BL = B // N_CORES          # 512 batch rows per core
NG = 64                    # groups per element type
H = 128                    # hidden width
NS = 8                     # e_s / e_b width

F32 = mybir.dt.float32
AF = mybir.ActivationFunctionType
OP = mybir.AluOpType

_ELEM = ("el", "kin")      # element_grad types
_FWD = ("dmp", "dual")     # forward-only types


def _declare_inputs(nc):
    aps = {}

    def din(name, shape):
        aps[name] = nc.dram_tensor(name, list(shape), F32, kind="ExternalInput").ap()

    din("xq", (NG, BL))
    din("xv", (NG, BL))
    din("esT", (NS, BL))
    din("ebT", (NS, BL))
    for t in _ELEM + _FWD:
        din(f"W1x_{t}", (NG, NG * H))
        din(f"W2T_{t}", (H, NG * H))
        din(f"b1c_{t}", (H, NG))
        din(f"b2c_{t}", (H, NG))
    for t in _ELEM:
        din(f"W2p_{t}", (H, NG * H))
        din(f"w1c_{t}", (H, NG))
        din(f"nw1c_{t}", (H, NG))
    for t in _FWD:
        din(f"w3c_{t}", (H, NG))
        din(f"b3c_{t}", (NG, 1))
    for nm in ("Bm2d", "Bk2g", "Cfk_m", "Cfk_g", "Cfm_k", "Cfm_d"):
        din(nm, (NG, NG))
    for nm in ("Bb2d", "Bs2g", "Cfk_b", "Cfm_s"):
        din(nm, (NS, NG))
    return aps


def build_nc():
    nc = bacc.Bacc("TRN2", target_bir_lowering=False, debug=False)
    aps = _declare_inputs(nc)
    fT = nc.dram_tensor("fT", [H, BL], F32, kind="ExternalOutput").ap()

    with tile.TileContext(nc) as tc, ExitStack() as ctx:
        const = ctx.enter_context(tc.tile_pool(name="const", bufs=1))
        wpool = ctx.enter_context(tc.tile_pool(name="wpool", bufs=6))
        work = ctx.enter_context(tc.tile_pool(name="work", bufs=6))
        state = ctx.enter_context(tc.tile_pool(name="state", bufs=1))
        psA = ctx.enter_context(tc.tile_pool(name="psA", bufs=2, space="PSUM"))
        psB = ctx.enter_context(tc.tile_pool(name="psB", bufs=2, space="PSUM"))
        psC = ctx.enter_context(tc.tile_pool(name="psC", bufs=2, space="PSUM"))
        psAcc = ctx.enter_context(tc.tile_pool(name="psAcc", bufs=2, space="PSUM"))

        # resident constants
        cst = {}
        for nm, ap in aps.items():
            if nm.startswith(("W1x_", "W2T_", "W2p_")):
                continue
            c = const.tile(list(ap.shape), F32, name=f"c_{nm}", tag=f"c_{nm}")
            nc.sync.dma_start(c, ap)
            cst[nm] = c

        ones_buf = const.tile([H, 2 * NG + NG], F32, name="ones_buf")
        nc.vector.memset(ones_buf, 0.0)
        nc.vector.memset(ones_buf[:, NG:NG + 1], 1.0)

        def ones_win(g):
            return ones_buf[:, NG - g:2 * NG - g]

        def elem_grad(t, xsrc, outT):
            """outT[g, b] = d/dx sum(group_mlp) for element type t."""
            w1x_d, w2t_d, w2p_d = aps[f"W1x_{t}"], aps[f"W2T_{t}"], aps[f"W2p_{t}"]
            b1c, b2c = cst[f"b1c_{t}"], cst[f"b2c_{t}"]
            w1c, nw1c = cst[f"w1c_{t}"], cst[f"nw1c_{t}"]
            pacc = psAcc.tile([NG, BL], F32, tag="acc", name=f"acc_{t}")
            h1s, h2s, g2s, g1ws, rps, pCs = {}, {}, {}, {}, {}, {}
            for step in range(NG + 4):
                if step < NG:
                    g = step
                    w1x = wpool.tile([NG, H], F32, tag="w1x", name=f"w1x_{t}{g}")
                    nc.sync.dma_start(w1x, w1x_d[:, g * H:(g + 1) * H])
                    w2t = wpool.tile([H, H], F32, tag="w2t", name=f"w2t_{t}{g}")
                    nc.sync.dma_start(w2t, w2t_d[:, g * H:(g + 1) * H])
                    w2p = wpool.tile([H, H], F32, tag="w2p", name=f"w2p_{t}{g}")
                    nc.sync.dma_start(w2p, w2p_d[:, g * H:(g + 1) * H])
                    pA = psA.tile([H, BL], F32, tag="pA", name=f"pA_{t}{g}")
                    nc.tensor.matmul(pA, w1x, xsrc, start=True, stop=True,
                                     skip_group_check=True)
                    h1 = work.tile([H, BL], F32, tag="h1", name=f"h1_{t}{g}")
                    nc.scalar.activation(h1, pA, AF.Tanh, bias=b1c[:, g:g + 1])
                    h1s[g] = h1
                    h1s[f"w2t{g}"] = w2t
                    h1s[f"w2p{g}"] = w2p
                if 1 <= step and step - 1 < NG:
                    g = step - 1
                    pB = psB.tile([H, BL], F32, tag="pB", name=f"pB_{t}{g}")
                    nc.tensor.matmul(pB, h1s[f"w2t{g}"], h1s[g], start=True,
                                     stop=True, skip_group_check=True)
                    h2 = work.tile([H, BL], F32, tag="h2", name=f"h2_{t}{g}")
                    nc.scalar.activation(h2, pB, AF.Tanh, bias=b2c[:, g:g + 1])
                    h2s[g] = h2
                if 2 <= step and step - 2 < NG:
                    g = step - 2
                    h1, h2 = h1s[g], h2s[g]
                    sq2 = work.tile([H, BL], F32, tag="sq2", name=f"sq2_{t}{g}")
                    nc.gpsimd.tensor_mul(sq2, h2, h2)
                    g2 = work.tile([H, BL], F32, tag="g2", name=f"g2_{t}{g}")
                    nc.vector.tensor_scalar(g2, sq2, -1.0, 1.0, OP.mult, OP.add)
                    g2s[g] = g2
                    sq1 = work.tile([H, BL], F32, tag="sq1", name=f"sq1_{t}{g}")
                    nc.gpsimd.tensor_mul(sq1, h1, h1)
                    g1w = work.tile([H, BL], F32, tag="g1w", name=f"g1w_{t}{g}")
                    nc.vector.tensor_scalar(g1w, sq1, nw1c[:, g:g + 1],
                                            w1c[:, g:g + 1], OP.mult, OP.add)
                    g1ws[g] = g1w
                    del h2s[g]
                if 3 <= step and step - 3 < NG:
                    g = step - 3
                    pC = psC.tile([H, BL], F32, tag="pC", name=f"pC_{t}{g}")
                    nc.tensor.matmul(pC, h1s[f"w2p{g}"], g2s[g], start=True,
                                     stop=True, skip_group_check=True)
                    rp = work.tile([H, BL], F32, tag="rp", name=f"rp_{t}{g}")
                    nc.vector.tensor_mul(rp, pC, g1ws[g])
                    rps[g] = rp
                    del g2s[g], g1ws[g], h1s[g]
                    del h1s[f"w2t{g}"], h1s[f"w2p{g}"]
                if 4 <= step:
                    g = step - 4
                    nc.tensor.matmul(pacc, ones_win(g), rps[g], start=(g == 0),
                                     stop=(g == NG - 1), skip_group_check=True)
                    del rps[g]
            nc.vector.tensor_copy(outT, pacc)

        def fwd_mlp(t, fsrcT, outT):
            """outT[g, b] = group_mlp(fsrc)[b, g]  (forward only)."""
            w1x_d, w2t_d = aps[f"W1x_{t}"], aps[f"W2T_{t}"]
            b1c, b2c = cst[f"b1c_{t}"], cst[f"b2c_{t}"]
            w3c, b3c = cst[f"w3c_{t}"], cst[f"b3c_{t}"]
            pacc = psAcc.tile([NG, BL], F32, tag="acc", name=f"acc_{t}")
            h1s, h2s = {}, {}
            for step in range(NG + 2):
                if step < NG:
                    g = step
                    w1x = wpool.tile([NG, H], F32, tag="w1x", name=f"w1x_{t}{g}")
                    nc.sync.dma_start(w1x, w1x_d[:, g * H:(g + 1) * H])
                    w2t = wpool.tile([H, H], F32, tag="w2t", name=f"w2t_{t}{g}")
                    nc.sync.dma_start(w2t, w2t_d[:, g * H:(g + 1) * H])
                    pA = psA.tile([H, BL], F32, tag="pA", name=f"pA_{t}{g}")
                    nc.tensor.matmul(pA, w1x, fsrcT, start=True, stop=True,
                                     skip_group_check=True)
                    h1 = work.tile([H, BL], F32, tag="h1", name=f"h1_{t}{g}")
                    nc.scalar.activation(h1, pA, AF.Tanh, bias=b1c[:, g:g + 1])
                    h1s[g] = h1
                    h1s[f"w2t{g}"] = w2t
                if 1 <= step and step - 1 < NG:
                    g = step - 1
                    pB = psB.tile([H, BL], F32, tag="pB", name=f"pB_{t}{g}")
                    nc.tensor.matmul(pB, h1s[f"w2t{g}"], h1s[g], start=True,
                                     stop=True, skip_group_check=True)
                    h2 = work.tile([H, BL], F32, tag="h2", name=f"h2_{t}{g}")
                    nc.scalar.activation(h2, pB, AF.Tanh, bias=b2c[:, g:g + 1])
                    h2s[g] = h2
                    del h1s[g], h1s[f"w2t{g}"]
                if 2 <= step:
                    g = step - 2
                    h2w = work.tile([H, BL], F32, tag="h2w", name=f"h2w_{t}{g}")
                    nc.vector.tensor_scalar_mul(h2w, h2s[g], w3c[:, g:g + 1])
                    nc.tensor.matmul(pacc, ones_win(g), h2w, start=(g == 0),
                                     stop=(g == NG - 1), skip_group_check=True)
                    del h2s[g]
            # outT = pacc + b3 (per-partition)
            nc.vector.tensor_scalar(outT, pacc, b3c[:, 0:1], None, OP.add)

        e_kT = state.tile([NG, BL], F32, name="e_kT")
        e_mT = state.tile([NG, BL], F32, name="e_mT")
        elem_grad("el", cst["xq"], e_kT)
        elem_grad("kin", cst["xv"], e_mT)

        # f_d^T = B_m2d^T-apply + B_b2d^T-apply ; f_g^T likewise
        pFD = psAcc.tile([NG, BL], F32, tag="acc", name="pFD")
        nc.tensor.matmul(pFD, cst["Bm2d"], e_mT, start=True, stop=False,
                         skip_group_check=True)
        nc.tensor.matmul(pFD, cst["Bb2d"], cst["ebT"], start=False, stop=True,
                         skip_group_check=True)
        f_dT = state.tile([NG, BL], F32, name="f_dT")
        nc.vector.tensor_copy(f_dT, pFD)

        pFG = psAcc.tile([NG, BL], F32, tag="acc", name="pFG")
        nc.tensor.matmul(pFG, cst["Bk2g"], e_kT, start=True, stop=False,
                         skip_group_check=True)
        nc.tensor.matmul(pFG, cst["Bs2g"], cst["esT"], start=False, stop=True,
                         skip_group_check=True)
        f_gT = state.tile([NG, BL], F32, name="f_gT")
        nc.vector.tensor_copy(f_gT, pFG)

        e_dT = state.tile([NG, BL], F32, name="e_dT")
        e_gT = state.tile([NG, BL], F32, name="e_gT")
        fwd_mlp("dmp", f_dT, e_dT)
        fwd_mlp("dual", f_gT, e_gT)

        # f_k^T = B_m2k^T-apply(e_m) + B_b2k^T-apply(e_b) - B_k2g-apply(e_g)
        pFK = psAcc.tile([NG, BL], F32, tag="acc", name="pFK")
        nc.tensor.matmul(pFK, cst["Cfk_m"], e_mT, start=True, stop=False,
                         skip_group_check=True)
        nc.tensor.matmul(pFK, cst["Cfk_b"], cst["ebT"], start=False, stop=False,
                         skip_group_check=True)
        nc.tensor.matmul(pFK, cst["Cfk_g"], e_gT, start=False, stop=True,
                         skip_group_check=True)
        fk = state.tile([NG, BL], F32, name="fk")
        nc.vector.tensor_copy(fk, pFK)
        nc.sync.dma_start(fT[0:NG, :], fk)

        pFM = psAcc.tile([NG, BL], F32, tag="acc", name="pFM")
        nc.tensor.matmul(pFM, cst["Cfm_k"], e_kT, start=True, stop=False,
                         skip_group_check=True)
        nc.tensor.matmul(pFM, cst["Cfm_s"], cst["esT"], start=False, stop=False,
                         skip_group_check=True)
        nc.tensor.matmul(pFM, cst["Cfm_d"], e_dT, start=False, stop=True,
                         skip_group_check=True)
        fm = state.tile([NG, BL], F32, name="fm")
        nc.vector.tensor_copy(fm, pFM)
        nc.sync.dma_start(fT[NG:H, :], fm)

    nc.compile()
    return nc


def make_weight_map(p_el, p_kin, p_dmp, p_dual,
                    B_k2g, B_m2d, B_s2g, B_b2d, B_m2k, B_b2k, B_s2m):
    """Host-side layout prep for all replicated (weight) tensors."""
    f32 = lambda a: np.ascontiguousarray(np.asarray(a, dtype=np.float32))
    wm = {}
    for t, p in (("el", p_el), ("kin", p_kin), ("dmp", p_dmp), ("dual", p_dual)):
        W1, b1 = f32(p["W1"]), f32(p["b1"])          # [G,H], [G,H]
        W2, b2 = f32(p["W2"]), f32(p["b2"])          # [G,H,H], [G,H]
        W3, b3 = f32(p["W3"]), f32(p["b3"])          # [G,H], [G]
        w1x = np.zeros((NG, NG * H), np.float32)
        for g in range(NG):
            w1x[g, g * H:(g + 1) * H] = W1[g]
        wm[f"W1x_{t}"] = w1x
        # W2T[j, g*H+i] = W2[g,i,j]
        wm[f"W2T_{t}"] = f32(W2.transpose(2, 0, 1).reshape(H, NG * H))
        wm[f"b1c_{t}"] = f32(b1.T)
        wm[f"b2c_{t}"] = f32(b2.T)
        if t in _ELEM:
            # W2p[i, g*H+j] = W2[g,i,j] * W3[g,i]
            wm[f"W2p_{t}"] = f32((W2 * W3[:, :, None]).transpose(1, 0, 2)
                                 .reshape(H, NG * H))
            wm[f"w1c_{t}"] = f32(W1.T)
            wm[f"nw1c_{t}"] = f32(-W1.T)
        else:
            wm[f"w3c_{t}"] = f32(W3.T)
            wm[f"b3c_{t}"] = f32(b3[:, None])
    B_k2g, B_m2d = f32(B_k2g), f32(B_m2d)
    B_s2g, B_b2d = f32(B_s2g), f32(B_b2d)
    B_m2k, B_b2k, B_s2m = f32(B_m2k), f32(B_b2k), f32(B_s2m)
    wm["Bm2d"] = B_m2d
    wm["Bb2d"] = B_b2d
    wm["Bk2g"] = B_k2g
    wm["Bs2g"] = B_s2g
    wm["Cfk_m"] = B_m2k
    wm["Cfk_b"] = B_b2k
    wm["Cfk_g"] = f32(-B_k2g.T)
    wm["Cfm_k"] = f32(-B_m2k.T)
    wm["Cfm_s"] = B_s2m
    wm["Cfm_d"] = f32(-B_m2d.T)
    return wm


def make_in_maps(x, e_s, e_b, wm):
    x = np.asarray(x, dtype=np.float32)
    e_s = np.asarray(e_s, dtype=np.float32)
    e_b = np.asarray(e_b, dtype=np.float32)
    in_maps = []
    for c in range(N_CORES):
        sl = slice(c * BL, (c + 1) * BL)
        m = dict(wm)
        m["xq"] = np.ascontiguousarray(x[sl, :NG].T)
        m["xv"] = np.ascontiguousarray(x[sl, NG:2 * NG].T)
        m["esT"] = np.ascontiguousarray(e_s[sl].T)
        m["ebT"] = np.ascontiguousarray(e_b[sl].T)
        in_maps.append(m)
    return in_maps


_NC_CACHE = {}


def kernel(x, e_s, e_b, p_el, p_kin, p_dmp, p_dual,
           B_k2g, B_m2d, B_s2g, B_b2d, B_m2k, B_b2k, B_s2m, **run_kwargs):
    if "nc" not in _NC_CACHE:
        _NC_CACHE["nc"] = build_nc()
    nc = _NC_CACHE["nc"]
    wm = make_weight_map(p_el, p_kin, p_dmp, p_dual,
                         B_k2g, B_m2d, B_s2g, B_b2d, B_m2k, B_b2k, B_s2m)
    in_maps = make_in_maps(x, e_s, e_b, wm)
    res = run_bass_kernel_spmd(nc, in_maps, core_ids=list(range(N_CORES)),
                               **run_kwargs)
    out = np.empty((B, H), dtype=np.float32)
    for c in range(N_CORES):
        out[c * BL:(c + 1) * BL, :] = res.results[c]["fT"].T
    kernel.last_results = res
    return out
